# revision 3
# baseline (speedup 1.0000x reference)
"""Multi-head causal attention (B=4, S=2048, D=1024, H=16) on 8 Trainium2 cores.

Sharding: core c -> (batch b = c//2, head-half hh = c%2), i.e. each core computes
attention for one batch and 8 of the 16 heads, plus the partial output
projection against its row-shard of Wo. Host sums the per-batch core pair
(the Wo row-shard all-reduce) and transposes.

On-device layout (per core, all matmul operands bf16, accumulation fp32):
  - q/k projections produce qT/kT [head-pair 128, S] (features on partitions)
  - scores are computed transposed: S^T[t, s] tiles (keys on partitions) so
    exp() on ScalarE writes P^T directly, and softmax denominators come for
    free from a ones-column appended to V during the P^T @ V_aug matmul.
  - causal masking: tiles strictly above the diagonal are never computed;
    diagonal tiles get a 0/1 mask multiply post-exp.
"""

import os
import sys
from contextlib import ExitStack

for _p in (
    "/opt/trn_rl_repo/concourse",
    "/root/.axon_site/_ro/trn_rl_repo/concourse",
):
    if os.path.isdir(_p) and _p not in sys.path:
        sys.path.append(_p)

import numpy as np
import ml_dtypes

BF16 = ml_dtypes.bfloat16

HD = 64          # head dim
NH = 8           # heads per core
G = NH // 2      # head-pair groups (2 heads -> 128 partitions)
EC = NH * HD // 128  # o^T feature chunks (=4)


def build_nc(S, D):
    import concourse.tile as tile
    from concourse import bacc, mybir

    f32 = mybir.dt.float32
    bf16 = mybir.dt.bfloat16
    Exp = mybir.ActivationFunctionType.Exp
    add = mybir.AluOpType.add
    mult = mybir.AluOpType.mult

    KC = D // 128    # contraction chunks over model dim
    ST = S // 128    # 128-token tiles
    SC = S // 512    # 512-token chunks

    nc = bacc.Bacc(None, target_bir_lowering=False)

    xq = nc.dram_tensor("xq", [D, S], bf16, kind="ExternalInput")
    xk = nc.dram_tensor("xk", [D, S], bf16, kind="ExternalInput")
    xv = nc.dram_tensor("xv", [D, S], bf16, kind="ExternalInput")
    wq = nc.dram_tensor("wq", [D, NH * HD], bf16, kind="ExternalInput")
    wk = nc.dram_tensor("wk", [D, NH * HD], bf16, kind="ExternalInput")
    wv = nc.dram_tensor("wv", [D, NH * HD], bf16, kind="ExternalInput")
    wo = nc.dram_tensor("wo", [NH * HD, D], bf16, kind="ExternalInput")
    bqd = nc.dram_tensor("bq", [128, G], f32, kind="ExternalInput")
    bkd = nc.dram_tensor("bk", [128, G], f32, kind="ExternalInput")
    bvd = nc.dram_tensor("bv", [128, NH, HD], f32, kind="ExternalInput")
    bod = nc.dram_tensor("bo", [128, D // 128], f32, kind="ExternalInput")
    maskd = nc.dram_tensor("mask", [128, 128], bf16, kind="ExternalInput")
    out = nc.dram_tensor("out", [D, S], f32, kind="ExternalOutput")

    with tile.TileContext(nc) as tc, ExitStack() as ctx:
        const_pool = ctx.enter_context(tc.tile_pool(name="const", bufs=1))
        xpool = ctx.enter_context(tc.tile_pool(name="x", bufs=4))
        qk_pool = ctx.enter_context(tc.tile_pool(name="qk", bufs=1))
        v_pool = ctx.enter_context(tc.tile_pool(name="v", bufs=1))
        pt_pool = ctx.enter_context(tc.tile_pool(name="pt", bufs=3))
        o_pool = ctx.enter_context(tc.tile_pool(name="o", bufs=1))
        r_pool = ctx.enter_context(tc.tile_pool(name="r", bufs=4))
        out_pool = ctx.enter_context(tc.tile_pool(name="outp", bufs=4))
        ps_mm = ctx.enter_context(tc.tile_pool(name="psmm", bufs=4, space="PSUM"))
        ps_st = ctx.enter_context(tc.tile_pool(name="psst", bufs=2, space="PSUM"))

        # ---- constants into SBUF ----
        wq_sb = const_pool.tile([128, KC, NH * HD], bf16)
        nc.sync.dma_start(wq_sb[:], wq.rearrange("(kc p) m -> p kc m", p=128))
        wk_sb = const_pool.tile([128, KC, NH * HD], bf16)
        nc.sync.dma_start(wk_sb[:], wk.rearrange("(kc p) m -> p kc m", p=128))
        wv_sb = const_pool.tile([128, KC, NH * HD], bf16)
        nc.sync.dma_start(wv_sb[:], wv.rearrange("(kc p) m -> p kc m", p=128))
        wo_sb = const_pool.tile([128, EC, D], bf16)
        nc.sync.dma_start(wo_sb[:], wo.rearrange("(ec p) d -> p ec d", p=128))
        bq_sb = const_pool.tile([128, G], f32)
        nc.sync.dma_start(bq_sb[:], bqd[:])
        bk_sb = const_pool.tile([128, G], f32)
        nc.sync.dma_start(bk_sb[:], bkd[:])
        bv_sb = const_pool.tile([128, NH, HD], f32)
        nc.sync.dma_start(bv_sb[:], bvd[:])
        bo_sb = const_pool.tile([128, D // 128], f32)
        nc.sync.dma_start(bo_sb[:], bod[:])
        mask_sb = const_pool.tile([128, 128], bf16)
        nc.sync.dma_start(mask_sb[:], maskd[:])

        qT = qk_pool.tile([128, G, S], bf16, tag="qT")
        kT = qk_pool.tile([128, G, S], bf16, tag="kT")
        v_sb = v_pool.tile([128, ST, NH, HD + 1], bf16, tag="v")
        oT = o_pool.tile([128, EC, S], bf16, tag="oT")

        # ones columns of V_aug (softmax denominator trick)
        nc.vector.memset(v_sb[:, :, :, HD : HD + 1], 1.0)

        # ---- Q and K projections: qT/kT[g] = (W[:, 2 heads].T @ x^T) + b ----
        for name, xsrc, wsb, bsb, dst in (
            ("q", xq, wq_sb, bq_sb, qT),
            ("k", xk, wk_sb, bk_sb, kT),
        ):
            for sc in range(SC):
                psums = [ps_mm.tile([128, 512], f32, tag="mm", name=f"ps{name}{sc}_{i}") for i in range(G)]
                for kc in range(KC):
                    xt = xpool.tile([128, 512], bf16, tag="xt")
                    nc.sync.dma_start(
                        xt[:], xsrc[kc * 128 : (kc + 1) * 128, sc * 512 : (sc + 1) * 512]
                    )
                    for g in range(G):
                        nc.tensor.matmul(
                            psums[g][:],
                            wsb[:, kc, g * 128 : (g + 1) * 128],
                            xt[:],
                            start=(kc == 0),
                            stop=(kc == KC - 1),
                        )
                for g in range(G):
                    # copy + per-partition bias on ScalarE
                    nc.scalar.add(
                        dst[:, g, sc * 512 : (sc + 1) * 512],
                        psums[g][:],
                        bsb[:, g : g + 1],
                    )

        # ---- V projection: v[s-tile] = x^T.T @ Wv  (+ bias, broadcast) ----
        for sc in range(SC):
            psums = [ps_mm.tile([128, NH, HD], f32, tag="mm", name=f"psv{sc}_{i}") for i in range(4)]
            for kc in range(KC):
                xt = xpool.tile([128, 512], bf16, tag="xt")
                nc.sync.dma_start(
                    xt[:], xv[kc * 128 : (kc + 1) * 128, sc * 512 : (sc + 1) * 512]
                )
                for i in range(4):
                    nc.tensor.matmul(
                        psums[i][:],
                        xt[:, i * 128 : (i + 1) * 128],
                        wv_sb[:, kc, :],
                        start=(kc == 0),
                        stop=(kc == KC - 1),
                    )
            for i in range(4):
                st = sc * 4 + i
                nc.vector.tensor_tensor(
                    v_sb[:, st, :, 0:HD], psums[i][:], bv_sb[:], add
                )

        # ---- attention per head ----
        for h in range(NH):
            g = h // 2
            ro = (h % 2) * HD  # partition row offset of this head in qT/kT
            qh = qT[ro : ro + HD, g, :]
            kh = kT[ro : ro + HD, g, :]

            av = [ps_mm.tile([128, 512], f32, tag="mm", name=f"av{h}_{i}") for i in range(SC)]

            for t in range(ST):
                s0 = t * 128
                N = S - s0
                pt = pt_pool.tile([128, S], bf16, tag="pt")
                # S^T tiles + exp, in chunks of 1024
                for c in range(0, N, 1024):
                    cn = min(1024, N - c)
                    stt = ps_st.tile([128, 1024], f32, tag="st")
                    for cc in range(0, cn, 512):
                        w = min(512, cn - cc)
                        nc.tensor.matmul(
                            stt[:, cc : cc + w],
                            kh[:, s0 : s0 + 128],
                            qh[:, s0 + c + cc : s0 + c + cc + w],
                            start=True,
                            stop=True,
                        )
                    nc.scalar.activation(
                        pt[:, c : c + cn], stt[:, 0:cn], Exp, scale=1.0 / np.sqrt(HD)
                    )
                # causal mask on the diagonal tile
                nc.vector.tensor_tensor(
                    pt[:, 0:128], pt[:, 0:128], mask_sb[:], mult
                )
                # accumulate O^T_aug[s-chunk] += V_aug[t].T @ P^T[t, s-chunk]
                for g4 in range(t * 128 // 512, SC):
                    lo = max(g4 * 512, s0)
                    n = (g4 + 1) * 512 - lo
                    nc.tensor.matmul(
                        av[g4][0 : HD + 1, lo - g4 * 512 : lo - g4 * 512 + n],
                        v_sb[:, t, h, :],
                        pt[:, lo - s0 : lo - s0 + n],
                        start=(t == 0),
                        stop=(t == (g4 * 4 + 3)),
                    )

            # normalize: o^T = o^T_unnorm * (1/rowsum), broadcast over features
            for g4 in range(SC):
                r1 = r_pool.tile([1, 512], f32, tag="r1")
                nc.vector.reciprocal(r1[:], av[g4][HD : HD + 1, :])
                r64 = r_pool.tile([HD, 512], f32, tag="r64")
                nc.gpsimd.partition_broadcast(r64[:], r1[:])
                nc.vector.tensor_tensor(
                    oT[ro : ro + HD, g, g4 * 512 : (g4 + 1) * 512],
                    av[g4][0:HD, :],
                    r64[:],
                    mult,
                )

        # ---- output projection: out = (Wo_shard.T @ o^T) + bo/2 ----
        for dc in range(D // 128):
            psums = [ps_mm.tile([128, 512], f32, tag="mm", name=f"pso{dc}_{i}") for i in range(SC)]
            for ec in range(EC):
                for sc in range(SC):
                    nc.tensor.matmul(
                        psums[sc][:],
                        wo_sb[:, ec, dc * 128 : (dc + 1) * 128],
                        oT[:, ec, sc * 512 : (sc + 1) * 512],
                        start=(ec == 0),
                        stop=(ec == EC - 1),
                    )
            for sc in range(SC):
                ot = out_pool.tile([128, 512], f32, tag="ot")
                nc.vector.tensor_scalar_add(ot[:], psums[sc][:], bo_sb[:, dc : dc + 1])
                nc.sync.dma_start(
                    out[dc * 128 : (dc + 1) * 128, sc * 512 : (sc + 1) * 512], ot[:]
                )

    nc.compile()
    return nc


def core_inputs(queries, keys, values, Wq, bq, Wk, bk, Wv, bv, Wo, bo, b, hh):
    """Build the per-core input map (host-side sharding + bf16 cast)."""
    D = queries.shape[2]
    hs = slice(hh * NH, hh * NH + NH)

    def xt(x):
        return np.ascontiguousarray(x[b].astype(BF16).T)

    def wcat(W):
        return np.ascontiguousarray(
            np.transpose(W[hs], (1, 0, 2)).reshape(D, NH * HD).astype(BF16)
        )

    def bstack(bias):
        return np.ascontiguousarray(
            bias[hs].reshape(G, 128).T.astype(np.float32)
        )

    mask = np.triu(np.ones((128, 128), np.float32)).astype(BF16)
    return {
        "xq": xt(queries),
        "xk": xt(keys),
        "xv": xt(values),
        "wq": wcat(Wq),
        "wk": wcat(Wk),
        "wv": wcat(Wv),
        "wo": np.ascontiguousarray(Wo[hh * NH * HD : (hh + 1) * NH * HD].astype(BF16)),
        "bq": bstack(bq),
        "bk": bstack(bk),
        "bv": np.ascontiguousarray(
            np.broadcast_to(bv[hs].reshape(1, NH, HD), (128, NH, HD)).astype(np.float32)
        ),
        "bo": np.ascontiguousarray(
            (bo.reshape(D // 128, 128) / 2.0).T.astype(np.float32)
        ),
        "mask": mask,
    }


_NC_CACHE = {}


def _get_nc(S, D):
    key = (S, D)
    if key not in _NC_CACHE:
        _NC_CACHE[key] = build_nc(S, D)
    return _NC_CACHE[key]


def kernel(keys, queries, values, Wq, bq, Wk, bk, Wv, bv, Wo, bo, _trace=False):
    keys, queries, values = (np.asarray(a) for a in (keys, queries, values))
    Wq, bq, Wk, bk, Wv, bv, Wo, bo = (
        np.asarray(a) for a in (Wq, bq, Wk, bk, Wv, bv, Wo, bo)
    )
    B, S, D = queries.shape
    nc = _get_nc(S, D)

    in_maps = [
        core_inputs(queries, keys, values, Wq, bq, Wk, bk, Wv, bv, Wo, bo, c // 2, c % 2)
        for c in range(8)
    ]
    from concourse.bass_utils import run_bass_kernel_spmd

    res = run_bass_kernel_spmd(
        nc, in_maps, core_ids=list(range(8)), trace=_trace
    )
    kernel.last_result = res
    outs = [r["out"] for r in res.results]
    out = np.empty((B, S, D), np.float32)
    for b in range(B):
        out[b] = (outs[2 * b] + outs[2 * b + 1]).T
    return out


# revision 5
# speedup vs baseline: 1.2592x; 1.2592x over previous
"""Multi-head causal attention (B=4, S=2048, D=1024, H=16) on 8 Trainium2 cores.

Sharding: core c -> (batch b = c//2, head-half hh = c%2), i.e. each core computes
attention for one batch and 8 of the 16 heads, plus the partial output
projection against its row-shard of Wo. Host sums the per-batch core pair
(the Wo row-shard all-reduce) and transposes.

On-device layout (per core, all matmul operands bf16, accumulation fp32):
  - q/k projections produce qT/kT [head-pair 128, S] (features on partitions)
  - scores are computed transposed: S^T[t, s] tiles (keys on partitions) so
    exp() on ScalarE writes P^T directly, and softmax denominators come for
    free from a ones-column appended to V during the P^T @ V_aug matmul.
  - causal masking: tiles strictly above the diagonal are never computed;
    diagonal tiles get a 0/1 mask multiply post-exp.
"""

import os
import sys
from contextlib import ExitStack

for _p in (
    "/opt/trn_rl_repo/concourse",
    "/root/.axon_site/_ro/trn_rl_repo/concourse",
):
    if os.path.isdir(_p) and _p not in sys.path:
        sys.path.append(_p)

import numpy as np
import ml_dtypes

BF16 = ml_dtypes.bfloat16

HD = 64          # head dim
NH = 8           # heads per core
G = NH // 2      # head-pair groups (2 heads -> 128 partitions)
EC = NH * HD // 128  # o^T feature chunks (=4)


def build_nc(S, D):
    import concourse.tile as tile
    from concourse import bacc, mybir

    f32 = mybir.dt.float32
    bf16 = mybir.dt.bfloat16
    Exp = mybir.ActivationFunctionType.Exp
    add = mybir.AluOpType.add
    mult = mybir.AluOpType.mult

    KC = D // 128    # contraction chunks over model dim
    ST = S // 128    # 128-token tiles
    SC = S // 512    # 512-token chunks

    nc = bacc.Bacc(None, target_bir_lowering=False)

    xq = nc.dram_tensor("xq", [D, S], bf16, kind="ExternalInput")
    xk = nc.dram_tensor("xk", [D, S], bf16, kind="ExternalInput")
    xv = nc.dram_tensor("xv", [D, S], bf16, kind="ExternalInput")
    wq = nc.dram_tensor("wq", [D, NH * HD], bf16, kind="ExternalInput")
    wk = nc.dram_tensor("wk", [D, NH * HD], bf16, kind="ExternalInput")
    wv = nc.dram_tensor("wv", [D, NH * HD], bf16, kind="ExternalInput")
    wo = nc.dram_tensor("wo", [NH * HD, D], bf16, kind="ExternalInput")
    bqd = nc.dram_tensor("bq", [128, G], f32, kind="ExternalInput")
    bkd = nc.dram_tensor("bk", [128, G], f32, kind="ExternalInput")
    bvd = nc.dram_tensor("bv", [128, NH, HD], f32, kind="ExternalInput")
    bod = nc.dram_tensor("bo", [128, D // 128], f32, kind="ExternalInput")
    maskd = nc.dram_tensor("mask", [128, 128], bf16, kind="ExternalInput")
    out = nc.dram_tensor("out", [D, S], f32, kind="ExternalOutput")

    with tile.TileContext(nc) as tc, ExitStack() as ctx:
        const_pool = ctx.enter_context(tc.tile_pool(name="const", bufs=1))
        xpool = ctx.enter_context(tc.tile_pool(name="x", bufs=4))
        qk_pool = ctx.enter_context(tc.tile_pool(name="qk", bufs=1))
        v_pool = ctx.enter_context(tc.tile_pool(name="v", bufs=1))
        pt_pool = ctx.enter_context(tc.tile_pool(name="pt", bufs=3))
        o_pool = ctx.enter_context(tc.tile_pool(name="o", bufs=1))
        r_pool = ctx.enter_context(tc.tile_pool(name="r", bufs=4))
        out_pool = ctx.enter_context(tc.tile_pool(name="outp", bufs=4))
        ps_mm = ctx.enter_context(tc.tile_pool(name="psmm", bufs=4, space="PSUM"))
        ps_st = ctx.enter_context(tc.tile_pool(name="psst", bufs=2, space="PSUM"))

        # ---- constants into SBUF ----
        wq_sb = const_pool.tile([128, KC, NH * HD], bf16)
        nc.sync.dma_start(wq_sb[:], wq.rearrange("(kc p) m -> p kc m", p=128))
        wk_sb = const_pool.tile([128, KC, NH * HD], bf16)
        nc.sync.dma_start(wk_sb[:], wk.rearrange("(kc p) m -> p kc m", p=128))
        wv_sb = const_pool.tile([128, KC, NH * HD], bf16)
        nc.sync.dma_start(wv_sb[:], wv.rearrange("(kc p) m -> p kc m", p=128))
        wo_sb = const_pool.tile([128, EC, D], bf16)
        nc.sync.dma_start(wo_sb[:], wo.rearrange("(ec p) d -> p ec d", p=128))
        bq_sb = const_pool.tile([128, G], f32)
        nc.sync.dma_start(bq_sb[:], bqd[:])
        bk_sb = const_pool.tile([128, G], f32)
        nc.sync.dma_start(bk_sb[:], bkd[:])
        bv_sb = const_pool.tile([128, NH, HD], f32)
        nc.sync.dma_start(bv_sb[:], bvd[:])
        bo_sb = const_pool.tile([128, D // 128], f32)
        nc.sync.dma_start(bo_sb[:], bod[:])
        mask_sb = const_pool.tile([128, 128], bf16)
        nc.sync.dma_start(mask_sb[:], maskd[:])

        qT = qk_pool.tile([128, G, S], bf16, tag="qT")
        kT = qk_pool.tile([128, G, S], bf16, tag="kT")
        v_sb = v_pool.tile([128, ST, NH, HD + 1], bf16, tag="v")
        oT = o_pool.tile([128, EC, S], bf16, tag="oT")

        # ones columns of V_aug (softmax denominator trick)
        nc.vector.memset(v_sb[:, :, :, HD : HD + 1], 1.0)

        # ---- Q and K projections: qT/kT[g] = (W[:, 2 heads].T @ x^T) + b ----
        for name, xsrc, wsb, bsb, dst in (
            ("q", xq, wq_sb, bq_sb, qT),
            ("k", xk, wk_sb, bk_sb, kT),
        ):
            for sc in range(SC):
                psums = [ps_mm.tile([128, 512], f32, tag="mm", name=f"ps{name}{sc}_{i}") for i in range(G)]
                for kc in range(KC):
                    xt = xpool.tile([128, 512], bf16, tag="xt")
                    nc.sync.dma_start(
                        xt[:], xsrc[kc * 128 : (kc + 1) * 128, sc * 512 : (sc + 1) * 512]
                    )
                    for g in range(G):
                        nc.tensor.matmul(
                            psums[g][:],
                            wsb[:, kc, g * 128 : (g + 1) * 128],
                            xt[:],
                            start=(kc == 0),
                            stop=(kc == KC - 1),
                        )
                for g in range(G):
                    # copy + per-partition bias on ScalarE
                    nc.scalar.add(
                        dst[:, g, sc * 512 : (sc + 1) * 512],
                        psums[g][:],
                        bsb[:, g : g + 1],
                    )

        # ---- V projection: v[s-tile] = x^T.T @ Wv  (+ bias, broadcast) ----
        for sc in range(SC):
            psums = [ps_mm.tile([128, NH, HD], f32, tag="mm", name=f"psv{sc}_{i}") for i in range(4)]
            for kc in range(KC):
                xt = xpool.tile([128, 512], bf16, tag="xt")
                nc.sync.dma_start(
                    xt[:], xv[kc * 128 : (kc + 1) * 128, sc * 512 : (sc + 1) * 512]
                )
                for i in range(4):
                    nc.tensor.matmul(
                        psums[i][:],
                        xt[:, i * 128 : (i + 1) * 128],
                        wv_sb[:, kc, :],
                        start=(kc == 0),
                        stop=(kc == KC - 1),
                    )
            for i in range(4):
                st = sc * 4 + i
                nc.vector.tensor_tensor(
                    v_sb[:, st, :, 0:HD], psums[i][:], bv_sb[:], add
                )

        # ---- attention per head ----
        for h in range(NH):
            g = h // 2
            ro = (h % 2) * HD  # partition row offset of this head in qT/kT
            qh = qT[ro : ro + HD, g, :]
            kh = kT[ro : ro + HD, g, :]

            # Phase A: all S^T tiles + exp for this head (P^T persists per t)
            pts = []
            for t in range(ST):
                s0 = t * 128
                N = S - s0
                pt = pt_pool.tile([128, N], bf16, tag=f"pt{t}", name=f"pt{h}_{t}", bufs=1)
                pts.append(pt)
                for c in range(0, N, 1024):
                    cn = min(1024, N - c)
                    stt = ps_st.tile([128, 1024], f32, tag="st", name=f"st{h}_{t}_{c}")
                    for cc in range(0, cn, 512):
                        w = min(512, cn - cc)
                        nc.tensor.matmul(
                            stt[:, cc : cc + w],
                            kh[:, s0 : s0 + 128],
                            qh[:, s0 + c + cc : s0 + c + cc + w],
                            start=True,
                            stop=True,
                        )
                    nc.scalar.activation(
                        pt[:, c : c + cn], stt[:, 0:cn], Exp, scale=1.0 / np.sqrt(HD)
                    )
                # causal mask on the diagonal tile
                nc.vector.tensor_tensor(
                    pt[:, 0:128], pt[:, 0:128], mask_sb[:], mult
                )

            # Phase B: AV accumulation, t-outer so P^T[t] is released early
            av = [ps_mm.tile([128, 512], f32, tag="mm", name=f"av{h}_{i}") for i in range(SC)]
            for t in range(ST):
                s0 = t * 128
                for g4 in range(t * 128 // 512, SC):
                    lo = max(g4 * 512, s0)
                    n = (g4 + 1) * 512 - lo
                    nc.tensor.matmul(
                        av[g4][0 : HD + 1, lo - g4 * 512 : lo - g4 * 512 + n],
                        v_sb[:, t, h, :],
                        pts[t][:, lo - s0 : lo - s0 + n],
                        start=(t == 0),
                        stop=(t == (g4 * 4 + 3)),
                    )
                    if t == g4 * 4 + 3:
                        # normalize o^T = o^T_unnorm * (1/rowsum) as soon as
                        # this s-group's accumulation completes
                        r1 = r_pool.tile([1, 512], f32, tag="r1", name=f"r1_{h}_{g4}")
                        nc.vector.reciprocal_approx_fast(r1[:], av[g4][HD : HD + 1, :])
                        r64 = r_pool.tile([HD, 512], f32, tag="r64", name=f"r64_{h}_{g4}")
                        nc.gpsimd.partition_broadcast(r64[:], r1[:])
                        nc.vector.tensor_tensor(
                            oT[ro : ro + HD, g, g4 * 512 : (g4 + 1) * 512],
                            av[g4][0:HD, :],
                            r64[:],
                            mult,
                        )

        # ---- output projection: out = (Wo_shard.T @ o^T) + bo/2 ----
        for dc in range(D // 128):
            psums = [ps_mm.tile([128, 512], f32, tag="mm", name=f"pso{dc}_{i}") for i in range(SC)]
            for ec in range(EC):
                for sc in range(SC):
                    nc.tensor.matmul(
                        psums[sc][:],
                        wo_sb[:, ec, dc * 128 : (dc + 1) * 128],
                        oT[:, ec, sc * 512 : (sc + 1) * 512],
                        start=(ec == 0),
                        stop=(ec == EC - 1),
                    )
            for sc in range(SC):
                ot = out_pool.tile([128, 512], f32, tag="ot")
                nc.vector.tensor_scalar_add(ot[:], psums[sc][:], bo_sb[:, dc : dc + 1])
                nc.sync.dma_start(
                    out[dc * 128 : (dc + 1) * 128, sc * 512 : (sc + 1) * 512], ot[:]
                )

    nc.compile()
    return nc


def core_inputs(queries, keys, values, Wq, bq, Wk, bk, Wv, bv, Wo, bo, b, hh):
    """Build the per-core input map (host-side sharding + bf16 cast)."""
    D = queries.shape[2]
    hs = slice(hh * NH, hh * NH + NH)

    def xt(x):
        return np.ascontiguousarray(x[b].astype(BF16).T)

    def wcat(W):
        return np.ascontiguousarray(
            np.transpose(W[hs], (1, 0, 2)).reshape(D, NH * HD).astype(BF16)
        )

    def bstack(bias):
        return np.ascontiguousarray(
            bias[hs].reshape(G, 128).T.astype(np.float32)
        )

    mask = np.triu(np.ones((128, 128), np.float32)).astype(BF16)
    return {
        "xq": xt(queries),
        "xk": xt(keys),
        "xv": xt(values),
        "wq": wcat(Wq),
        "wk": wcat(Wk),
        "wv": wcat(Wv),
        "wo": np.ascontiguousarray(Wo[hh * NH * HD : (hh + 1) * NH * HD].astype(BF16)),
        "bq": bstack(bq),
        "bk": bstack(bk),
        "bv": np.ascontiguousarray(
            np.broadcast_to(bv[hs].reshape(1, NH, HD), (128, NH, HD)).astype(np.float32)
        ),
        "bo": np.ascontiguousarray(
            (bo.reshape(D // 128, 128) / 2.0).T.astype(np.float32)
        ),
        "mask": mask,
    }


_NC_CACHE = {}


def _get_nc(S, D):
    key = (S, D)
    if key not in _NC_CACHE:
        _NC_CACHE[key] = build_nc(S, D)
    return _NC_CACHE[key]


def kernel(keys, queries, values, Wq, bq, Wk, bk, Wv, bv, Wo, bo, _trace=False):
    keys, queries, values = (np.asarray(a) for a in (keys, queries, values))
    Wq, bq, Wk, bk, Wv, bv, Wo, bo = (
        np.asarray(a) for a in (Wq, bq, Wk, bk, Wv, bv, Wo, bo)
    )
    B, S, D = queries.shape
    nc = _get_nc(S, D)

    in_maps = [
        core_inputs(queries, keys, values, Wq, bq, Wk, bk, Wv, bv, Wo, bo, c // 2, c % 2)
        for c in range(8)
    ]
    from concourse.bass_utils import run_bass_kernel_spmd

    res = run_bass_kernel_spmd(
        nc, in_maps, core_ids=list(range(8)), trace=_trace
    )
    kernel.last_result = res
    outs = [r["out"] for r in res.results]
    out = np.empty((B, S, D), np.float32)
    for b in range(B):
        out[b] = (outs[2 * b] + outs[2 * b + 1]).T
    return out


# revision 6
# speedup vs baseline: 1.3516x; 1.0734x over previous
"""Multi-head causal attention (B=4, S=2048, D=1024, H=16) on 8 Trainium2 cores.

Sharding: core c -> (batch b = c//2, head-half hh = c%2), i.e. each core computes
attention for one batch and 8 of the 16 heads, plus the partial output
projection against its row-shard of Wo. Host sums the per-batch core pair
(the Wo row-shard all-reduce) and transposes.

On-device layout (per core, all matmul operands bf16, accumulation fp32):
  - q/k projections produce qT/kT [head-pair 128, S] (features on partitions)
  - scores are computed transposed: S^T[t, s] tiles (keys on partitions) so
    exp() on ScalarE writes P^T directly, and softmax denominators come for
    free from a ones-column appended to V during the P^T @ V_aug matmul.
  - causal masking: tiles strictly above the diagonal are never computed;
    diagonal tiles get a 0/1 mask multiply post-exp.
"""

import os
import sys
from contextlib import ExitStack

for _p in (
    "/opt/trn_rl_repo/concourse",
    "/root/.axon_site/_ro/trn_rl_repo/concourse",
):
    if os.path.isdir(_p) and _p not in sys.path:
        sys.path.append(_p)

import numpy as np
import ml_dtypes

BF16 = ml_dtypes.bfloat16

HD = 64          # head dim
NH = 8           # heads per core
G = NH // 2      # head-pair groups (2 heads -> 128 partitions)
EC = NH * HD // 128  # o^T feature chunks (=4)


def build_nc(S, D):
    import concourse.tile as tile
    from concourse import bacc, mybir

    f32 = mybir.dt.float32
    bf16 = mybir.dt.bfloat16
    Exp = mybir.ActivationFunctionType.Exp
    add = mybir.AluOpType.add
    mult = mybir.AluOpType.mult

    KC = D // 128    # contraction chunks over model dim
    ST = S // 128    # 128-token tiles
    SC = S // 512    # 512-token chunks

    nc = bacc.Bacc(None, target_bir_lowering=False)

    xq = nc.dram_tensor("xq", [D, S], bf16, kind="ExternalInput")
    xk = nc.dram_tensor("xk", [D, S], bf16, kind="ExternalInput")
    xv = nc.dram_tensor("xv", [D, S], bf16, kind="ExternalInput")
    wq = nc.dram_tensor("wq", [D, NH * HD], bf16, kind="ExternalInput")
    wk = nc.dram_tensor("wk", [D, NH * HD], bf16, kind="ExternalInput")
    wv = nc.dram_tensor("wv", [D, NH * HD], bf16, kind="ExternalInput")
    wo = nc.dram_tensor("wo", [NH * HD, D], bf16, kind="ExternalInput")
    bqd = nc.dram_tensor("bq", [128, G], f32, kind="ExternalInput")
    bkd = nc.dram_tensor("bk", [128, G], f32, kind="ExternalInput")
    bvd = nc.dram_tensor("bv", [128, NH, HD], f32, kind="ExternalInput")
    bod = nc.dram_tensor("bo", [128, D // 128], f32, kind="ExternalInput")
    maskd = nc.dram_tensor("mask", [128, 128], bf16, kind="ExternalInput")
    out = nc.dram_tensor("out", [D, S], f32, kind="ExternalOutput")

    with tile.TileContext(nc) as tc, ExitStack() as ctx:
        const_pool = ctx.enter_context(tc.tile_pool(name="const", bufs=1))
        xpool = ctx.enter_context(tc.tile_pool(name="x", bufs=4))
        qk_pool = ctx.enter_context(tc.tile_pool(name="qk", bufs=1))
        v_pool = ctx.enter_context(tc.tile_pool(name="v", bufs=1))
        pt_pool = ctx.enter_context(tc.tile_pool(name="pt", bufs=3))
        o_pool = ctx.enter_context(tc.tile_pool(name="o", bufs=1))
        r_pool = ctx.enter_context(tc.tile_pool(name="r", bufs=4))
        out_pool = ctx.enter_context(tc.tile_pool(name="outp", bufs=4))
        ps_mm = ctx.enter_context(tc.tile_pool(name="psmm", bufs=4, space="PSUM"))
        ps_st = ctx.enter_context(tc.tile_pool(name="psst", bufs=2, space="PSUM"))

        # ---- constants into SBUF ----
        wq_sb = const_pool.tile([128, KC, NH * HD], bf16)
        nc.sync.dma_start(wq_sb[:], wq.rearrange("(kc p) m -> p kc m", p=128))
        wk_sb = const_pool.tile([128, KC, NH * HD], bf16)
        nc.sync.dma_start(wk_sb[:], wk.rearrange("(kc p) m -> p kc m", p=128))
        wv_sb = const_pool.tile([128, KC, NH * HD], bf16)
        nc.sync.dma_start(wv_sb[:], wv.rearrange("(kc p) m -> p kc m", p=128))
        wo_sb = const_pool.tile([128, EC, D], bf16)
        nc.sync.dma_start(wo_sb[:], wo.rearrange("(ec p) d -> p ec d", p=128))
        bq_sb = const_pool.tile([128, G], f32)
        nc.sync.dma_start(bq_sb[:], bqd[:])
        bk_sb = const_pool.tile([128, G], f32)
        nc.sync.dma_start(bk_sb[:], bkd[:])
        bv_sb = const_pool.tile([128, NH, HD], f32)
        nc.sync.dma_start(bv_sb[:], bvd[:])
        bo_sb = const_pool.tile([128, D // 128], f32)
        nc.sync.dma_start(bo_sb[:], bod[:])
        mask_sb = const_pool.tile([128, 128], bf16)
        nc.sync.dma_start(mask_sb[:], maskd[:])

        qT = qk_pool.tile([128, G, S], bf16, tag="qT")
        kT = qk_pool.tile([128, G, S], bf16, tag="kT")
        v_sb = v_pool.tile([128, ST, NH, HD + 1], bf16, tag="v")
        oT = o_pool.tile([128, EC, S], bf16, tag="oT")

        # ones columns of V_aug (softmax denominator trick)
        nc.vector.memset(v_sb[:, :, :, HD : HD + 1], 1.0)

        # ---- Q and K projections: qT/kT[g] = (W[:, 2 heads].T @ x^T) + b ----
        for name, xsrc, wsb, bsb, dst in (
            ("q", xq, wq_sb, bq_sb, qT),
            ("k", xk, wk_sb, bk_sb, kT),
        ):
            for sc in range(SC):
                psums = [ps_mm.tile([128, 512], f32, tag="mm", name=f"ps{name}{sc}_{i}") for i in range(G)]
                for kc in range(KC):
                    xt = xpool.tile([128, 512], bf16, tag="xt")
                    nc.sync.dma_start(
                        xt[:], xsrc[kc * 128 : (kc + 1) * 128, sc * 512 : (sc + 1) * 512]
                    )
                    for g in range(G):
                        nc.tensor.matmul(
                            psums[g][:],
                            wsb[:, kc, g * 128 : (g + 1) * 128],
                            xt[:],
                            start=(kc == 0),
                            stop=(kc == KC - 1),
                        )
                for g in range(G):
                    # copy + per-partition bias (DVE; ScalarE is saturated by exp)
                    nc.vector.tensor_scalar_add(
                        dst[:, g, sc * 512 : (sc + 1) * 512],
                        psums[g][:],
                        bsb[:, g : g + 1],
                    )

        # ---- attention: software-pipelined over heads ----
        # emit_st(h): S^T tiles + exp + mask for head h  (PE: 40 MMs, ACT: exps)
        # emit_av(h): AV accumulation + normalization    (PE: 40 MMs)
        # Emission order: st(0) | Vproj | {st(1), av(0)} | {st(2), av(1)} | ...
        # st(h+1) tile t is interleaved with av(h) tile t so the in-order PE
        # queue always has ready work while ACT catches up on exps.
        pts_all = [None] * NH

        def emit_st_tile(h, t, pts):
            g = h // 2
            ro = (h % 2) * HD
            qh = qT[ro : ro + HD, g, :]
            kh = kT[ro : ro + HD, g, :]
            s0 = t * 128
            N = S - s0
            pt = pt_pool.tile([128, N], bf16, tag=f"pt{t}_{h % 2}", name=f"pt{h}_{t}", bufs=1)
            pts.append(pt)
            for c in range(0, N, 1024):
                cn = min(1024, N - c)
                stt = ps_st.tile([128, 1024], f32, tag="st", name=f"st{h}_{t}_{c}")
                for cc in range(0, cn, 512):
                    w = min(512, cn - cc)
                    nc.tensor.matmul(
                        stt[:, cc : cc + w],
                        kh[:, s0 : s0 + 128],
                        qh[:, s0 + c + cc : s0 + c + cc + w],
                        start=True,
                        stop=True,
                    )
                nc.scalar.activation(
                    pt[:, c : c + cn], stt[:, 0:cn], Exp, scale=1.0 / np.sqrt(HD)
                )
            # causal mask on the diagonal tile
            nc.vector.tensor_tensor(pt[:, 0:128], pt[:, 0:128], mask_sb[:], mult)

        def emit_av_tile(h, t, av, pts):
            g = h // 2
            ro = (h % 2) * HD
            s0 = t * 128
            for g4 in range(t * 128 // 512, SC):
                lo = max(g4 * 512, s0)
                n = (g4 + 1) * 512 - lo
                nc.tensor.matmul(
                    av[g4][0 : HD + 1, lo - g4 * 512 : lo - g4 * 512 + n],
                    v_sb[:, t, h, :],
                    pts[t][:, lo - s0 : lo - s0 + n],
                    start=(t == 0),
                    stop=(t == (g4 * 4 + 3)),
                )
                if t == g4 * 4 + 3:
                    # normalize o^T = o^T_unnorm * (1/rowsum) once this
                    # s-group's accumulation completes
                    r1 = r_pool.tile([1, 512], f32, tag="r1", name=f"r1_{h}_{g4}")
                    nc.vector.reciprocal_approx_fast(r1[:], av[g4][HD : HD + 1, :])
                    r64 = r_pool.tile([HD, 512], f32, tag="r64", name=f"r64_{h}_{g4}")
                    nc.gpsimd.partition_broadcast(r64[:], r1[:])
                    nc.vector.tensor_tensor(
                        oT[ro : ro + HD, g, g4 * 512 : (g4 + 1) * 512],
                        av[g4][0:HD, :],
                        r64[:],
                        mult,
                    )

        # head 0's scores can start as soon as Q/K projections are done
        pts_all[0] = []
        for t in range(ST):
            emit_st_tile(0, t, pts_all[0])

        # ---- V projection: v[s-tile] = x^T.T @ Wv  (+ bias, broadcast) ----
        for sc in range(SC):
            psums = [ps_mm.tile([128, NH, HD], f32, tag="mm", name=f"psv{sc}_{i}") for i in range(4)]
            for kc in range(KC):
                xt = xpool.tile([128, 512], bf16, tag="xt")
                nc.sync.dma_start(
                    xt[:], xv[kc * 128 : (kc + 1) * 128, sc * 512 : (sc + 1) * 512]
                )
                for i in range(4):
                    nc.tensor.matmul(
                        psums[i][:],
                        xt[:, i * 128 : (i + 1) * 128],
                        wv_sb[:, kc, :],
                        start=(kc == 0),
                        stop=(kc == KC - 1),
                    )
            for i in range(4):
                st = sc * 4 + i
                nc.vector.tensor_tensor(
                    v_sb[:, st, :, 0:HD], psums[i][:], bv_sb[:], add
                )

        for h in range(NH):
            av = [ps_mm.tile([128, 512], f32, tag="mm", name=f"av{h}_{i}") for i in range(SC)]
            if h + 1 < NH:
                pts_all[h + 1] = []
            for t in range(ST):
                if h + 1 < NH:
                    emit_st_tile(h + 1, t, pts_all[h + 1])
                emit_av_tile(h, t, av, pts_all[h])
            pts_all[h] = None

        # ---- output projection: out = (Wo_shard.T @ o^T) + bo/2 ----
        for dc in range(D // 128):
            psums = [ps_mm.tile([128, 512], f32, tag="mm", name=f"pso{dc}_{i}") for i in range(SC)]
            for ec in range(EC):
                for sc in range(SC):
                    nc.tensor.matmul(
                        psums[sc][:],
                        wo_sb[:, ec, dc * 128 : (dc + 1) * 128],
                        oT[:, ec, sc * 512 : (sc + 1) * 512],
                        start=(ec == 0),
                        stop=(ec == EC - 1),
                    )
            for sc in range(SC):
                ot = out_pool.tile([128, 512], f32, tag="ot")
                nc.vector.tensor_scalar_add(ot[:], psums[sc][:], bo_sb[:, dc : dc + 1])
                nc.sync.dma_start(
                    out[dc * 128 : (dc + 1) * 128, sc * 512 : (sc + 1) * 512], ot[:]
                )

    nc.compile()
    return nc


def core_inputs(queries, keys, values, Wq, bq, Wk, bk, Wv, bv, Wo, bo, b, hh):
    """Build the per-core input map (host-side sharding + bf16 cast)."""
    D = queries.shape[2]
    hs = slice(hh * NH, hh * NH + NH)

    def xt(x):
        return np.ascontiguousarray(x[b].astype(BF16).T)

    def wcat(W):
        return np.ascontiguousarray(
            np.transpose(W[hs], (1, 0, 2)).reshape(D, NH * HD).astype(BF16)
        )

    def bstack(bias):
        return np.ascontiguousarray(
            bias[hs].reshape(G, 128).T.astype(np.float32)
        )

    mask = np.triu(np.ones((128, 128), np.float32)).astype(BF16)
    return {
        "xq": xt(queries),
        "xk": xt(keys),
        "xv": xt(values),
        "wq": wcat(Wq),
        "wk": wcat(Wk),
        "wv": wcat(Wv),
        "wo": np.ascontiguousarray(Wo[hh * NH * HD : (hh + 1) * NH * HD].astype(BF16)),
        "bq": bstack(bq),
        "bk": bstack(bk),
        "bv": np.ascontiguousarray(
            np.broadcast_to(bv[hs].reshape(1, NH, HD), (128, NH, HD)).astype(np.float32)
        ),
        "bo": np.ascontiguousarray(
            (bo.reshape(D // 128, 128) / 2.0).T.astype(np.float32)
        ),
        "mask": mask,
    }


_NC_CACHE = {}


def _get_nc(S, D):
    key = (S, D)
    if key not in _NC_CACHE:
        _NC_CACHE[key] = build_nc(S, D)
    return _NC_CACHE[key]


def kernel(keys, queries, values, Wq, bq, Wk, bk, Wv, bv, Wo, bo, _trace=False):
    keys, queries, values = (np.asarray(a) for a in (keys, queries, values))
    Wq, bq, Wk, bk, Wv, bv, Wo, bo = (
        np.asarray(a) for a in (Wq, bq, Wk, bk, Wv, bv, Wo, bo)
    )
    B, S, D = queries.shape
    nc = _get_nc(S, D)

    in_maps = [
        core_inputs(queries, keys, values, Wq, bq, Wk, bk, Wv, bv, Wo, bo, c // 2, c % 2)
        for c in range(8)
    ]
    from concourse.bass_utils import run_bass_kernel_spmd

    res = run_bass_kernel_spmd(
        nc, in_maps, core_ids=list(range(8)), trace=_trace
    )
    kernel.last_result = res
    outs = [r["out"] for r in res.results]
    out = np.empty((B, S, D), np.float32)
    for b in range(B):
        out[b] = (outs[2 * b] + outs[2 * b + 1]).T
    return out


# revision 7
# speedup vs baseline: 1.4657x; 1.0844x over previous
"""Multi-head causal attention (B=4, S=2048, D=1024, H=16) on 8 Trainium2 cores.

Sharding: core c -> (batch b = c//2, head-half hh = c%2), i.e. each core computes
attention for one batch and 8 of the 16 heads, plus the partial output
projection against its row-shard of Wo. Host sums the per-batch core pair
(the Wo row-shard all-reduce) and transposes.

On-device layout (per core, all matmul operands bf16, accumulation fp32):
  - q/k projections produce qT/kT [head-pair 128, S] (features on partitions)
  - scores are computed transposed: S^T[t, s] tiles (keys on partitions) so
    exp() on ScalarE writes P^T directly, and softmax denominators come for
    free from a ones-column appended to V during the P^T @ V_aug matmul.
  - causal masking: tiles strictly above the diagonal are never computed;
    diagonal tiles get a 0/1 mask multiply post-exp.
  - the two heads of a pair live on disjoint partition rows (0-63 / 64-127),
    so their K=64 S^T matmuls execute concurrently in the PE array
    (row-group tiling); attention is pipelined over (pair, s-range) units.
"""

import os
import sys
from contextlib import ExitStack

for _p in (
    "/opt/trn_rl_repo/concourse",
    "/root/.axon_site/_ro/trn_rl_repo/concourse",
):
    if os.path.isdir(_p) and _p not in sys.path:
        sys.path.append(_p)

import numpy as np
import ml_dtypes

BF16 = ml_dtypes.bfloat16

HD = 64          # head dim
NH = 8           # heads per core
G = NH // 2      # head-pair groups (2 heads -> 128 partitions)
EC = NH * HD // 128  # o^T feature chunks (=4)


def build_nc(S, D):
    import concourse.tile as tile
    from concourse import bacc, mybir

    f32 = mybir.dt.float32
    bf16 = mybir.dt.bfloat16
    Exp = mybir.ActivationFunctionType.Exp
    add = mybir.AluOpType.add
    mult = mybir.AluOpType.mult

    KC = D // 128    # contraction chunks over model dim
    ST = S // 128    # 128-token tiles
    SC = S // 512    # 512-token score groups
    NU = max(1, SC // 2)          # s-range units per head pair (1024 cols each)
    UW = (SC // NU) * 512         # unit width in columns

    nc = bacc.Bacc(None, target_bir_lowering=False)

    xq = nc.dram_tensor("xq", [D, S], bf16, kind="ExternalInput")
    xk = nc.dram_tensor("xk", [D, S], bf16, kind="ExternalInput")
    xv = nc.dram_tensor("xv", [D, S], bf16, kind="ExternalInput")
    wq = nc.dram_tensor("wq", [D, NH * HD], bf16, kind="ExternalInput")
    wk = nc.dram_tensor("wk", [D, NH * HD], bf16, kind="ExternalInput")
    wv = nc.dram_tensor("wv", [D, NH * HD], bf16, kind="ExternalInput")
    wo = nc.dram_tensor("wo", [NH * HD, D], bf16, kind="ExternalInput")
    bqd = nc.dram_tensor("bq", [128, G], f32, kind="ExternalInput")
    bkd = nc.dram_tensor("bk", [128, G], f32, kind="ExternalInput")
    bvd = nc.dram_tensor("bv", [128, NH, HD], f32, kind="ExternalInput")
    bod = nc.dram_tensor("bo", [128, D // 128], f32, kind="ExternalInput")
    maskd = nc.dram_tensor("mask", [128, 128], bf16, kind="ExternalInput")
    out = nc.dram_tensor("out", [D, S], f32, kind="ExternalOutput")

    with tile.TileContext(nc) as tc, ExitStack() as ctx:
        const_pool = ctx.enter_context(tc.tile_pool(name="const", bufs=1))
        qk_pool = ctx.enter_context(tc.tile_pool(name="qk", bufs=1))
        v_pool = ctx.enter_context(tc.tile_pool(name="v", bufs=1))
        pt_pool = ctx.enter_context(tc.tile_pool(name="pt", bufs=1))
        o_pool = ctx.enter_context(tc.tile_pool(name="o", bufs=1))
        r_pool = ctx.enter_context(tc.tile_pool(name="r", bufs=4))
        out_pool = ctx.enter_context(tc.tile_pool(name="outp", bufs=4))
        ps_mm = ctx.enter_context(tc.tile_pool(name="psmm", bufs=4, space="PSUM"))
        ps_st = ctx.enter_context(tc.tile_pool(name="psst", bufs=2, space="PSUM"))

        # ---- constants into SBUF ----
        wo_sb = const_pool.tile([128, EC, D], bf16)
        nc.sync.dma_start(wo_sb[:], wo.rearrange("(ec p) d -> p ec d", p=128))
        bq_sb = const_pool.tile([128, G], f32)
        nc.sync.dma_start(bq_sb[:], bqd[:])
        bk_sb = const_pool.tile([128, G], f32)
        nc.sync.dma_start(bk_sb[:], bkd[:])
        bv_sb = const_pool.tile([128, NH, HD], f32)
        nc.sync.dma_start(bv_sb[:], bvd[:])
        bo_sb = const_pool.tile([128, D // 128], f32)
        nc.sync.dma_start(bo_sb[:], bod[:])
        mask_sb = const_pool.tile([128, 128], bf16)
        nc.sync.dma_start(mask_sb[:], maskd[:])

        qT = qk_pool.tile([128, G, S], bf16, tag="qT")
        kT = qk_pool.tile([128, G, S], bf16, tag="kT")
        v_sb = v_pool.tile([128, ST, NH, HD + 1], bf16, tag="v")
        oT = o_pool.tile([128, EC, S], bf16, tag="oT")

        # ones columns of V_aug (softmax denominator trick)
        nc.vector.memset(v_sb[:, :, :, HD : HD + 1], 1.0)

        # ---- units: (head-pair g, s-range u) ----
        units = [(g, u) for g in range(G) for u in range(NU)]

        def unit_geom(u):
            s_lo, s_hi = u * UW, min((u + 1) * UW, S)
            ts, offs, cols = [], {}, 0
            for t in range(0, s_hi // 128):
                w = s_hi - max(t * 128, s_lo)
                if w <= 0:
                    continue
                ts.append(t)
                offs[t] = cols
                cols += w
            return s_lo, s_hi, ts, offs, cols

        def st_unit(g, u):
            """S^T + exp + mask for both heads of pair g over unit u's columns.
            The two heads' K=64 matmuls use disjoint PE row groups and run
            concurrently."""
            s_lo, s_hi, ts, offs, cols = unit_geom(u)
            pts = []
            for j in range(2):
                pt = pt_pool.tile(
                    [128, cols], bf16, tag=f"ph{u}_{j}", name=f"pt{g}_{u}_{j}", bufs=1
                )
                pts.append(pt)
            # chunk plan: walk t ranges, pack into 1024-col exp windows
            chunks = []  # (t, s_from, win, win_off, wlen)
            pos = 0
            for t in ts:
                s_from = max(t * 128, s_lo)
                rem = s_hi - s_from
                while rem:
                    wlen = min(512, rem, 1024 - pos % 1024)
                    chunks.append((t, s_from, pos // 1024, pos % 1024, wlen))
                    pos += wlen
                    s_from += wlen
                    rem -= wlen
            nwin = (pos + 1023) // 1024
            for w in range(nwin):
                wchunks = [c for c in chunks if c[2] == w]
                wcols = sum(c[4] for c in wchunks)
                wbase = wchunks[0][3] + 1024 * w - wchunks[0][3]  # global col base
                gbase = 1024 * w
                stt = []
                for j in range(2):
                    st_t = ps_st.tile(
                        [128, 1024], f32, tag="st", name=f"st{g}_{u}_{w}_{j}"
                    )
                    stt.append(st_t)
                for t, s_from, _, woff, wlen in wchunks:
                    for j in range(2):
                        ro = j * HD
                        nc.tensor.matmul(
                            stt[j][:, woff : woff + wlen],
                            kT[ro : ro + HD, g, t * 128 : t * 128 + 128],
                            qT[ro : ro + HD, g, s_from : s_from + wlen],
                            start=True,
                            stop=True,
                        )
                for j in range(2):
                    nc.scalar.activation(
                        pts[j][:, gbase : gbase + wcols],
                        stt[j][:, 0:wcols],
                        Exp,
                        scale=1.0 / np.sqrt(HD),
                    )
            # causal mask on diagonal tiles (t starting inside this unit)
            for t in ts:
                if t * 128 >= s_lo:
                    for j in range(2):
                        nc.vector.tensor_tensor(
                            pts[j][:, offs[t] : offs[t] + 128],
                            pts[j][:, offs[t] : offs[t] + 128],
                            mask_sb[:],
                            mult,
                        )
            return pts

        def av_unit(g, u, pts):
            """O^T accumulation + normalization for both heads of pair g,
            s-groups of unit u."""
            s_lo, s_hi, ts, offs, cols = unit_geom(u)
            g4s = range(s_lo // 512, s_hi // 512)
            av = {}
            for j in range(2):
                for g4 in g4s:
                    av[j, g4] = ps_mm.tile(
                        [128, 512], f32, tag="mm", name=f"av{g}_{u}_{j}_{g4}"
                    )
            for t in ts:
                for j in range(2):
                    h = 2 * g + j
                    ro = j * HD
                    for g4 in g4s:
                        if t * 128 >= (g4 + 1) * 512:
                            continue
                        lo = max(g4 * 512, t * 128)
                        n = (g4 + 1) * 512 - lo
                        col = offs[t] + lo - max(t * 128, s_lo)
                        nc.tensor.matmul(
                            av[j, g4][0 : HD + 1, lo - g4 * 512 : lo - g4 * 512 + n],
                            v_sb[:, t, h, :],
                            pts[j][:, col : col + n],
                            start=(t == 0),
                            stop=(t == g4 * 4 + 3),
                        )
                        if t == g4 * 4 + 3:
                            # o^T = o^T_unnorm * (1/rowsum)
                            r1 = r_pool.tile(
                                [1, 512], f32, tag="r1", name=f"r1_{g}_{u}_{j}_{g4}"
                            )
                            nc.vector.reciprocal_approx_fast(
                                r1[:], av[j, g4][HD : HD + 1, :]
                            )
                            r64 = r_pool.tile(
                                [HD, 512], f32, tag="r64", name=f"r64_{g}_{u}_{j}_{g4}"
                            )
                            nc.gpsimd.partition_broadcast(r64[:], r1[:])
                            nc.vector.tensor_tensor(
                                oT[ro : ro + HD, g, g4 * 512 : (g4 + 1) * 512],
                                av[j, g4][0:HD, :],
                                r64[:],
                                mult,
                            )

        # ---- projections (weight/activation pools released afterwards) ----
        with tc.tile_pool(name="wqkv", bufs=1) as wpool, tc.tile_pool(
            name="x", bufs=4
        ) as xpool:
            wq_sb = wpool.tile([128, KC, NH * HD], bf16, tag="wq")
            nc.sync.dma_start(wq_sb[:], wq.rearrange("(kc p) m -> p kc m", p=128))
            wk_sb = wpool.tile([128, KC, NH * HD], bf16, tag="wk")
            nc.sync.dma_start(wk_sb[:], wk.rearrange("(kc p) m -> p kc m", p=128))
            wv_sb = wpool.tile([128, KC, NH * HD], bf16, tag="wv")
            nc.sync.dma_start(wv_sb[:], wv.rearrange("(kc p) m -> p kc m", p=128))

            # Q and K projections: qT/kT[g] = (W[:, 2 heads].T @ x^T) + b
            for name, xsrc, wsb, bsb, dst in (
                ("q", xq, wq_sb, bq_sb, qT),
                ("k", xk, wk_sb, bk_sb, kT),
            ):
                for sc in range(SC):
                    psums = [
                        ps_mm.tile([128, 512], f32, tag="mm", name=f"ps{name}{sc}_{i}")
                        for i in range(G)
                    ]
                    for kc in range(KC):
                        xt = xpool.tile([128, 512], bf16, tag="xt")
                        nc.sync.dma_start(
                            xt[:],
                            xsrc[kc * 128 : (kc + 1) * 128, sc * 512 : (sc + 1) * 512],
                        )
                        for g in range(G):
                            nc.tensor.matmul(
                                psums[g][:],
                                wsb[:, kc, g * 128 : (g + 1) * 128],
                                xt[:],
                                start=(kc == 0),
                                stop=(kc == KC - 1),
                            )
                    for g in range(G):
                        # copy + per-partition bias (DVE; ScalarE is busy with exp)
                        nc.vector.tensor_scalar_add(
                            dst[:, g, sc * 512 : (sc + 1) * 512],
                            psums[g][:],
                            bsb[:, g : g + 1],
                        )

            # first unit's scores can start while V projection proceeds
            pts_next = st_unit(*units[0])

            # V projection: v[s-tile] = x^T.T @ Wv (+ bias, broadcast)
            for sc in range(SC):
                psums = [
                    ps_mm.tile([128, NH, HD], f32, tag="mm", name=f"psv{sc}_{i}")
                    for i in range(4)
                ]
                for kc in range(KC):
                    xt = xpool.tile([128, 512], bf16, tag="xt")
                    nc.sync.dma_start(
                        xt[:], xv[kc * 128 : (kc + 1) * 128, sc * 512 : (sc + 1) * 512]
                    )
                    for i in range(4):
                        nc.tensor.matmul(
                            psums[i][:],
                            xt[:, i * 128 : (i + 1) * 128],
                            wv_sb[:, kc, :],
                            start=(kc == 0),
                            stop=(kc == KC - 1),
                        )
                for i in range(4):
                    sti = sc * 4 + i
                    nc.vector.tensor_tensor(
                        v_sb[:, sti, :, 0:HD], psums[i][:], bv_sb[:], add
                    )

        # ---- attention pipeline over units ----
        for i, (g, u) in enumerate(units):
            pts_cur = pts_next
            if i + 1 < len(units):
                pts_next = st_unit(*units[i + 1])
            av_unit(g, u, pts_cur)

        # ---- output projection: out = (Wo_shard.T @ o^T) + bo/2 ----
        for dc in range(D // 128):
            psums = [
                ps_mm.tile([128, 512], f32, tag="mm", name=f"pso{dc}_{i}")
                for i in range(SC)
            ]
            for ec in range(EC):
                for sc in range(SC):
                    nc.tensor.matmul(
                        psums[sc][:],
                        wo_sb[:, ec, dc * 128 : (dc + 1) * 128],
                        oT[:, ec, sc * 512 : (sc + 1) * 512],
                        start=(ec == 0),
                        stop=(ec == EC - 1),
                    )
            for sc in range(SC):
                ot = out_pool.tile([128, 512], f32, tag="ot")
                nc.vector.tensor_scalar_add(ot[:], psums[sc][:], bo_sb[:, dc : dc + 1])
                nc.sync.dma_start(
                    out[dc * 128 : (dc + 1) * 128, sc * 512 : (sc + 1) * 512], ot[:]
                )

    nc.compile()
    return nc


def core_inputs(queries, keys, values, Wq, bq, Wk, bk, Wv, bv, Wo, bo, b, hh):
    """Build the per-core input map (host-side sharding + bf16 cast)."""
    D = queries.shape[2]
    hs = slice(hh * NH, hh * NH + NH)

    def xt(x):
        return np.ascontiguousarray(x[b].astype(BF16).T)

    def wcat(W):
        return np.ascontiguousarray(
            np.transpose(W[hs], (1, 0, 2)).reshape(D, NH * HD).astype(BF16)
        )

    def bstack(bias):
        return np.ascontiguousarray(
            bias[hs].reshape(G, 128).T.astype(np.float32)
        )

    mask = np.triu(np.ones((128, 128), np.float32)).astype(BF16)
    return {
        "xq": xt(queries),
        "xk": xt(keys),
        "xv": xt(values),
        "wq": wcat(Wq),
        "wk": wcat(Wk),
        "wv": wcat(Wv),
        "wo": np.ascontiguousarray(Wo[hh * NH * HD : (hh + 1) * NH * HD].astype(BF16)),
        "bq": bstack(bq),
        "bk": bstack(bk),
        "bv": np.ascontiguousarray(
            np.broadcast_to(bv[hs].reshape(1, NH, HD), (128, NH, HD)).astype(np.float32)
        ),
        "bo": np.ascontiguousarray(
            (bo.reshape(D // 128, 128) / 2.0).T.astype(np.float32)
        ),
        "mask": mask,
    }


_NC_CACHE = {}


def _get_nc(S, D):
    key = (S, D)
    if key not in _NC_CACHE:
        _NC_CACHE[key] = build_nc(S, D)
    return _NC_CACHE[key]


def kernel(keys, queries, values, Wq, bq, Wk, bk, Wv, bv, Wo, bo, _trace=False):
    keys, queries, values = (np.asarray(a) for a in (keys, queries, values))
    Wq, bq, Wk, bk, Wv, bv, Wo, bo = (
        np.asarray(a) for a in (Wq, bq, Wk, bk, Wv, bv, Wo, bo)
    )
    B, S, D = queries.shape
    nc = _get_nc(S, D)

    in_maps = [
        core_inputs(queries, keys, values, Wq, bq, Wk, bk, Wv, bv, Wo, bo, c // 2, c % 2)
        for c in range(8)
    ]
    from concourse.bass_utils import run_bass_kernel_spmd

    res = run_bass_kernel_spmd(
        nc, in_maps, core_ids=list(range(8)), trace=_trace
    )
    kernel.last_result = res
    outs = [r["out"] for r in res.results]
    out = np.empty((B, S, D), np.float32)
    for b in range(B):
        out[b] = (outs[2 * b] + outs[2 * b + 1]).T
    return out
